# revision 5
# baseline (speedup 1.0000x reference)
"""Locally-connected 2D layer on 8 Trainium2 NeuronCores.

Problem: x[128,3,64,64] f32, per-position weights W[60,60,32,75], bias b[60,60,32]
  out[b,o,y,x] = sum_k patches[b,y,x,k] * W[y,x,o,k] + b[y,x,o],  k=(c,dy,dx)

Strategy (spatial sharding over output rows, 8 rows/core, memory-regime):
  - Host pre-transposes x to xh[row, c, w, batch] (padded to 68 rows) so every
    on-device patch-plane copy is a single contiguous 30KB run.
  - The contraction (c,dy,dx)=75 must live on SBUF partitions for the PE. dy is
    handled with a mod-5 ring of "patch planes" XP[(c, r%5, dx), x*128+b]; the
    per-row dy rotation is folded into the HOST-side W layout (np.roll), so the
    device always reads XP[0:76] with one contiguous partition range.
  - Bias is folded in as contraction row 75 (W row 75 = bias, XP row 75 = 1.0).
  - Per output row: 15 groups of 4 column-tiled matmuls (lhsT=W[76,32],
    rhs=XP[76,128] -> out[32o,128b] at PSUM partitions 32j), PSUM->SBUF via DVE,
    DMA out in a DMA-friendly layout; host re-transposes the output once.
"""

import numpy as np

B, C, H, WIDTH = 128, 3, 64, 64
KH = KW = 5
RY = RX = 60
O = 32
K = 75
NCORES = 8
RPC = 8             # output rows computed per core (8*8=64, last 4 dropped)
INR = RPC + KH - 1  # 12 input rows per core
PADH = NCORES * RPC + KH - 1  # 68
NG = 15             # groups of 4 x-positions per row
CHUNKS = ((0, 4), (4, 4), (8, 4), (12, 3))  # (first group, n groups) per PSUM chunk

_cache = {}


def _build():
    import concourse.bass as bass
    import concourse.bacc as bacc
    import concourse.tile as tile
    import concourse.mybir as mybir

    f32 = mybir.dt.float32
    nc = bacc.Bacc("TRN2", target_bir_lowering=False, debug=False,
                   num_devices=NCORES)
    xh_d = nc.dram_tensor("xh", [INR, C, WIDTH, B], f32, kind="ExternalInput")
    wh_d = nc.dram_tensor("wh", [RPC, K + 1, RX, O], f32, kind="ExternalInput")
    ones_d = nc.dram_tensor("ones", [1, RX * B], f32, kind="ExternalInput")
    oc_d = nc.dram_tensor("oc", [RPC, 4, O, NG, B], f32, kind="ExternalOutput")

    FXB = RX * B  # 7680 elements per patch plane

    with tile.TileContext(nc) as tc:
        with (
            tc.tile_pool(name="const", bufs=1) as cpool,
            tc.tile_pool(name="w", bufs=2) as wpool,
            tc.tile_pool(name="os", bufs=3) as opool,
            tc.tile_pool(name="ps", bufs=3, space=bass.MemorySpace.PSUM) as ppool,
        ):
            xs = cpool.tile([INR * C, WIDTH * B], f32)  # [36, 8192]
            xp = cpool.tile([K + 1, FXB], f32)          # [76, 7680]

            nc.sync.dma_start(xp[K:K + 1, :], ones_d[:])

            xh_flat = xh_d[:].rearrange("r c w b -> (r c) (w b)")
            for r in range(INR):
                nc.sync.dma_start(xs[r * C:(r + 1) * C, :],
                                  xh_flat[r * C:(r + 1) * C, :])

            def fill(r):
                # copy input row r into ring slot r%5: 15 contiguous 30KB runs
                rm = r % KH
                for c in range(C):
                    for dx in range(KW):
                        p = c * (KH * KW) + rm * KW + dx
                        nc.gpsimd.dma_start(
                            xp[p:p + 1, 0:FXB],
                            xs[r * C + c:r * C + c + 1, dx * B:dx * B + FXB],
                        )

            for r in range(KH):
                fill(r)

            for k in range(RPC):
                wt = wpool.tile([K + 1, RX * O], f32)
                nc.sync.dma_start(wt[:], wh_d[k].rearrange("k x o -> k (x o)"))
                for (g0, gn) in CHUNKS:
                    pt = ppool.tile([128, 4 * B], f32)
                    for gs in range(gn):
                        for j in range(4):
                            xpos = (g0 + gs) * 4 + j
                            nc.tensor.matmul(
                                pt[32 * j:32 * (j + 1), gs * B:(gs + 1) * B],
                                wt[:, xpos * O:(xpos + 1) * O],
                                xp[:, xpos * B:(xpos + 1) * B],
                                tile_position=(0, 32 * j),
                            )
                    ot = opool.tile([128, 4 * B], f32)
                    nc.vector.tensor_copy(ot[:, :gn * B], pt[:, :gn * B])
                    dst = oc_d[k, :, :, g0:g0 + gn, :].rearrange(
                        "j o g b -> (j o) (g b)")
                    nc.scalar.dma_start(dst, ot[:, :gn * B])
                if k + KH < INR:
                    fill(k + KH)

    nc.compile()
    return nc


def _get_nc():
    if "nc" not in _cache:
        _cache["nc"] = _build()
    return _cache["nc"]


def _prep_inputs(x, W, b):
    x = np.asarray(x, np.float32)
    W = np.asarray(W, np.float32)
    b = np.asarray(b, np.float32)
    xh = np.zeros((PADH, C, WIDTH, B), np.float32)
    xh[:H] = x.transpose(2, 1, 3, 0)  # [H, C, W, B]
    Wfull = W.transpose(0, 3, 1, 2)   # [RY, K, RX, O]
    in_maps = []
    for i in range(NCORES):
        whc = np.zeros((RPC, K + 1, RX, O), np.float32)
        for k in range(RPC):
            y = RPC * i + k
            if y < RY:
                w5 = Wfull[y].reshape(C, KH, KW, RX, O)
                # device ring slot rm holds input row with (local row)%5 == rm;
                # slot rm supplies dy=(rm-k)%5 for output row k -> roll by k
                whc[k, :K] = np.roll(w5, k, axis=1).reshape(K, RX, O)
                whc[k, K] = b[y]
        in_maps.append({
            "xh": np.ascontiguousarray(xh[RPC * i:RPC * i + INR]),
            "wh": whc,
            "ones": np.ones((1, RX * B), np.float32),
        })
    return in_maps


def kernel(x, W, b):
    from concourse.bass_utils import run_bass_kernel_spmd

    nc = _get_nc()
    in_maps = _prep_inputs(x, W, b)
    br = run_bass_kernel_spmd(nc, in_maps, list(range(NCORES)),
                              **_cache.get("run_kwargs", {}))
    _cache["last_run"] = br
    oc = np.stack([np.asarray(br.results[i]["oc"]) for i in range(NCORES)])
    oc = oc.reshape(NCORES * RPC, 4, O, NG, B)  # [64, j, o, x4, b]
    out = oc.transpose(4, 2, 0, 3, 1).reshape(B, O, NCORES * RPC, RX)
    return np.ascontiguousarray(out[:, :, :RY, :])


# revision 12
# speedup vs baseline: 2.0836x; 2.0836x over previous
"""Locally-connected 2D layer on 8 Trainium2 NeuronCores.

Problem: x[128,3,64,64] f32, per-position weights W[60,60,32,75], bias b[60,60,32]
  out[b,o,y,x] = sum_k patches[b,y,x,k] * W[y,x,o,k] + b[y,x,o],  k=(c,dy,dx)

Strategy (spatial sharding over output rows, 8 rows/core, memory-regime):
  - The contraction (c,dy,dx)=75 must live on SBUF partitions for the PE. dy is
    handled with a mod-5 ring of "patch planes" XP[(r%5, c, dx), x*128+b]; the
    per-row dy rotation is folded into the HOST-side W layout (np.roll), so the
    device always reads XP[0:76] as one contiguous partition range.
  - Ring planes are pre-replicated on the HOST (dx-im2col) into xpr[12,15,FXB]
    so every device fill is a plain [15, 30KB] DRAM->SBUF slice copy; fills are
    split into 4 free-chunks gated on the matmul chunks that last read the
    slot, so the ring advance overlaps the row's own compute.
  - Bias is folded in as contraction row 75 (W row 75 = bias, XP row 75 = 1.0).
  - Per output row: 15 groups of 4 column-tiled matmuls (lhsT=W[76,32],
    rhs=XP[76,128] -> out[32o,128b] at PSUM partitions 32j), PSUM->SBUF via DVE,
    one 983KB store per row in a DMA-friendly layout; host re-transposes once.
"""

import numpy as np

B, C, H, WIDTH = 128, 3, 64, 64
KH = KW = 5
RY = RX = 60
O = 32
K = 75
NCORES = 8
RPC = 8             # output rows computed per core (8*8=64, last 4 dropped)
INR = RPC + KH - 1  # 12 input rows per core
PADH = NCORES * RPC + KH - 1  # 68
NG = 15             # groups of 4 x-positions per row
CHUNKS = ((0, 4), (4, 4), (8, 4), (12, 3))  # (first group, n groups) per PSUM chunk
FXB = RX * B        # 7680 elements per patch plane

_cache = {}


def _build():
    import concourse.bass as bass
    import concourse.bacc as bacc
    import concourse.tile as tile
    import concourse.mybir as mybir

    f32 = mybir.dt.float32
    nc = bacc.Bacc("TRN2", target_bir_lowering=False, debug=False,
                   num_devices=NCORES)
    xpr_d = nc.dram_tensor("xpr", [INR, KH * C, FXB], f32, kind="ExternalInput")
    wh_d = nc.dram_tensor("wh", [RPC, K + 1, RX, O], f32, kind="ExternalInput")
    ones_d = nc.dram_tensor("ones", [1, FXB], f32, kind="ExternalInput")
    oc_d = nc.dram_tensor("oc", [RPC, 4, O, NG, B], f32, kind="ExternalOutput")

    NPL = KH * C  # 15 planes per input row

    with tile.TileContext(nc) as tc:
        with (
            tc.tile_pool(name="const", bufs=1) as cpool,
            tc.tile_pool(name="w", bufs=4) as wpool,
            tc.tile_pool(name="os", bufs=2) as opool,
            tc.tile_pool(name="ps", bufs=4, space=bass.MemorySpace.PSUM) as ppool,
        ):
            xp = cpool.tile([K + 1, FXB], f32)  # [76, 7680]; row 75 = ones

            nc.sync.dma_start(xp[K:K + 1, :], ones_d[:])
            for r in range(KH):  # initial ring: rows 0..4 -> slots 0..4
                nc.gpsimd.dma_start(xp[r * NPL:(r + 1) * NPL, :], xpr_d[r])

            wts = {}

            def load_w(k):
                wts[k] = wpool.tile([K + 1, RX * O], f32, name="wt", tag="wt")
                nc.gpsimd.dma_start(wts[k][:],
                                    wh_d[k].rearrange("k x o -> k (x o)"))

            load_w(0)
            load_w(1)

            for k in range(RPC):
                wt = wts.pop(k)
                ot = opool.tile([128, NG * B], f32)  # [128, 1920]
                for ci, (g0, gn) in enumerate(CHUNKS):
                    pt = ppool.tile([128, 4 * B], f32)
                    for gs in range(gn):
                        for j in range(4):
                            xpos = (g0 + gs) * 4 + j
                            nc.tensor.matmul(
                                pt[32 * j:32 * (j + 1), gs * B:(gs + 1) * B],
                                wt[:, xpos * O:(xpos + 1) * O],
                                xp[:, xpos * B:(xpos + 1) * B],
                                tile_position=(0, 32 * j),
                            )
                    nc.vector.tensor_copy(
                        ot[:, g0 * B:(g0 + gn) * B], pt[:, :gn * B])
                    if k + KH < INR:
                        # ring advance for row k+1: overwrite slot k%5 with
                        # input row k+5, chunk-gated on this chunk's matmuls
                        slot = k % KH
                        f0, f1 = g0 * 4 * B, (g0 + gn) * 4 * B
                        nc.gpsimd.dma_start(
                            xp[slot * NPL:(slot + 1) * NPL, f0:f1],
                            xpr_d[k + KH, :, f0:f1])
                if k + 2 < RPC:
                    load_w(k + 2)
                nc.scalar.dma_start(
                    oc_d[k].rearrange("j o g b -> (j o) (g b)"), ot[:])

    nc.compile()
    return nc


def _get_nc():
    if "nc" not in _cache:
        _cache["nc"] = _build()
    return _cache["nc"]


def _prep_inputs(x, W, b):
    x = np.asarray(x, np.float32)
    W = np.asarray(W, np.float32)
    b = np.asarray(b, np.float32)
    xh = np.zeros((PADH, C, WIDTH, B), np.float32)
    xh[:H] = x.transpose(2, 1, 3, 0)  # [row, c, w, batch]
    # ring planes: xpr_full[r, (c,dx) -> c*KW+dx, x, b] = xh[r, c, x+dx, b]
    # plane order within a slot must be p2 = c*KW + dx (with slot-major rm)
    xpr_full = np.zeros((PADH, C, KW, RX, B), np.float32)
    for dx in range(KW):
        xpr_full[:, :, dx] = xh[:, :, dx:dx + RX]
    xpr_full = xpr_full.reshape(PADH, C * KW, FXB)
    Wfull = W.transpose(0, 3, 1, 2)  # [RY, K, RX, O]
    in_maps = []
    for i in range(NCORES):
        whc = np.zeros((RPC, K + 1, RX, O), np.float32)
        for k in range(RPC):
            y = RPC * i + k
            if y < RY:
                w5 = Wfull[y].reshape(C, KH, KW, RX, O)
                # device slot rm holds input row with (local row)%5 == rm;
                # slot rm supplies dy=(rm-k)%5 for output row k -> roll by k.
                # partition order: p = rm*15 + c*5 + dx
                whc[k, :K] = np.roll(w5, k, axis=1).transpose(1, 0, 2, 3, 4) \
                    .reshape(K, RX, O)
                whc[k, K] = b[y]
        in_maps.append({
            "xpr": np.ascontiguousarray(xpr_full[RPC * i:RPC * i + INR]),
            "wh": whc,
            "ones": np.ones((1, FXB), np.float32),
        })
    return in_maps


def kernel(x, W, b):
    from concourse.bass_utils import run_bass_kernel_spmd

    nc = _get_nc()
    in_maps = _prep_inputs(x, W, b)
    br = run_bass_kernel_spmd(nc, in_maps, list(range(NCORES)),
                              **_cache.get("run_kwargs", {}))
    _cache["last_run"] = br
    oc = np.stack([np.asarray(br.results[i]["oc"]) for i in range(NCORES)])
    oc = oc.reshape(NCORES * RPC, 4, O, NG, B)  # [64, j, o, x4, b]
    out = oc.transpose(4, 2, 0, 3, 1).reshape(B, O, NCORES * RPC, RX)
    return np.ascontiguousarray(out[:, :, :RY, :])
